# revision 1
# baseline (speedup 1.0000x reference)
"""Trainium2 Bass kernel for nn_DenseModel_51926154609008 (weighted-rank
contrastive CE loss).

Math (reference semantics, no sort needed):
  scores = q @ p.T                       [B=2048, P=16384]
  t_i    = scores[i, 8*i]                (positive/target score)
  rank_i = #{j : scores[i, j] > t_i}     (argsort position == exceed count,
                                          ties are measure-zero for randn data)
  lse_i  = logsumexp(scores[i, :])
  loss   = mean((lse_i - t_i) * (1 + 2.6*exp(-(rank_i-1)^2 / (2*1.8^2))))

Sharding: passage-parallel (P split across 8 cores, q replicated) — 12.6MB
of HBM reads per core vs 51MB for query-parallel with replicated passages.
Each core computes a [2048, 2048] score slab in 32 half-tiles
([128 queries x 1024 passages], one 2-bank PSUM buffer each) and reduces
every half-tile to per-query partials:
  sumexp_c[i] = sum_j exp(s_ij - C)      (fixed shift C so partials add
                                          across cores without a max-merge)
  cnt_c[i]    = #{j in slab : s_ij > t_i}
The host combines partials and evaluates the tiny [2048] tail in fp64.

The self-comparison (j == 8i) must contribute exactly 0 to rank_i. Query i's
target column lives only in core (i//256)'s slab. Each core rotates its query
order (data-level permutation — the program stays SPMD-uniform) so its own
queries always land on m-tiles OWN_M, OWN_M+1; the two half-tiles containing
self-columns use a masked count (indicator * mask, one fused DVE op); all
other half-tiles use a plain per-partition is_gt count.

t itself is computed on the host (trivial 2048x768 row-dot).

HW notes baked in from trace/bisect evidence:
  - DVE ops fault when an access pattern spans >2 PSUM banks; 2 banks is
    fine -> [128, 1024] half-tiles, one count op each.
  - ACT reads spanning 4 banks are fine; exp uses the per-instruction
    accumulator (sum along free dim) so no junk reduction is needed.
  - Mixing ACT functions (Exp/Sigmoid) forces ~1.3us ACT_TABLE_LOADs; the
    kernel uses Exp only.
  - bf16 matmuls stream at ~216ns per [128x512] MM warm; fp32 runs 2x
    slower and float32r ~1.9x (fp32_mode=HIGH, no FWL weight loads).
  - Input DMAs are split into [128, 512] sub-chunks, ordered so the first
    m-tile's operands land first (whole-tile DMAs starved the PE for ~14us).
"""

import sys

import numpy as np

sys.path.insert(0, "/opt/trn_rl_repo")

import concourse.bacc as bacc  # noqa: E402
import concourse.bass as bass  # noqa: E402
import concourse.mybir as mybir  # noqa: E402
import concourse.tile as tile  # noqa: E402
from concourse.bass_utils import run_bass_kernel_spmd  # noqa: E402

# Problem shape (hardcoded per the task contract).
B = 2048
D = 768
NP = 8
P = B * NP  # 16384
NCORES = 8
PSLAB = P // NCORES  # 2048 passage columns per core
KCH = D // 128  # 6 contraction chunks
MT = B // 128  # 16 query m-tiles
NU = 2 * MT  # 32 half-tile units of [128, 1024]
QSLAB = B // NCORES  # 256 queries owned per core
OWN_M = 8  # own queries sit at m-tiles 8,9 (mask off the critical path)

C_SHIFT = 128.0  # fixed exp shift: exp(s - C) never overflows for this data

ALPHA = 2.6
OPTIMAL_RANK = 1.0
SIGMA = 1.8

# Matmul input dtype: bfloat16 | float32r | float32
MM_DT = mybir.dt.bfloat16

_STATE: dict = {}


def _build_nc(mm_dt):
    nc = bacc.Bacc("TRN2", target_bir_lowering=False, debug=False,
                   num_devices=NCORES)

    qT_d = nc.dram_tensor("qT", [D, B], mm_dt, kind="ExternalInput").ap()
    pT_d = nc.dram_tensor("pT", [D, PSLAB], mm_dt, kind="ExternalInput").ap()
    tv_d = nc.dram_tensor("tvec", [128, MT], mybir.dt.float32,
                          kind="ExternalInput").ap()
    msk_d = nc.dram_tensor("msk", [128, 1024], mybir.dt.float32,
                           kind="ExternalInput").ap()
    se_d = nc.dram_tensor("se_out", [128, 2 * NU], mybir.dt.float32,
                          kind="ExternalOutput").ap()
    cnt_d = nc.dram_tensor("cnt_out", [128, 2 * NU], mybir.dt.float32,
                           kind="ExternalOutput").ap()

    f32 = mybir.dt.float32
    bf16 = mybir.dt.bfloat16

    with tile.TileContext(nc) as tc:
        with (
            tc.tile_pool(name="weights", bufs=1) as wpool,
            tc.tile_pool(name="stats", bufs=1) as spool,
            tc.tile_pool(name="junk", bufs=3) as jpool,
            tc.tile_pool(name="psum", bufs=4,
                         space=bass.MemorySpace.PSUM) as ppool,
        ):
            qk = [wpool.tile([128, B], mm_dt, name=f"qk{k}", tag=f"qk{k}")
                  for k in range(KCH)]
            pk = [wpool.tile([128, PSLAB], mm_dt, name=f"pk{k}", tag=f"pk{k}")
                  for k in range(KCH)]

            def ldq(k, part):  # issued on GpSimd's sequencer
                nc.gpsimd.dma_start(
                    qk[k][:, part * 512:(part + 1) * 512],
                    qT_d[k * 128:(k + 1) * 128, part * 512:(part + 1) * 512])

            def ldp(k, half):  # issued on Sync's sequencer
                nc.sync.dma_start(
                    pk[k][:, half * 1024:(half + 1) * 1024],
                    pT_d[k * 128:(k + 1) * 128, half * 1024:(half + 1) * 1024])

            # Units run nh-major (all half-0 m-tiles, then all half-1), so
            # pk half 1 isn't needed until mid-kernel; qk part p feeds
            # m-tiles 4p..4p+3. DMA issue is ~0.6us per dma_start on the
            # issuing sequencer, so the critical first operands go first,
            # split across two sequencers (Sync: pk, GpSimd: qk).
            tv = spool.tile([128, MT], f32, name="tv", tag="tv")
            msk = spool.tile([128, 1024], f32, name="msk", tag="msk")
            # smallest-possible first dependencies: MM#0 needs qk0 cols
            # 0:128 (LDWEIGHTS) and pk0 cols 0:512 only
            nc.gpsimd.dma_start(qk[0][:, 0:128], qT_d[0:128, 0:128])
            nc.sync.dma_start(pk[0][:, 0:512], pT_d[0:128, 0:512])
            nc.gpsimd.dma_start(qk[0][:, 128:512], qT_d[0:128, 128:512])
            nc.sync.dma_start(pk[0][:, 512:1024], pT_d[0:128, 512:1024])
            for k in range(1, KCH):
                ldq(k, 0)
                if k <= 3:
                    nc.sync.dma_start(pk[k][:, 0:1024],
                                      pT_d[k * 128:(k + 1) * 128, 0:1024])
                else:
                    # third sequencer so the k-chain of first-unit operands
                    # arrives faster than the PE consumes it
                    nc.scalar.dma_start(pk[k][:, 0:1024],
                                        pT_d[k * 128:(k + 1) * 128, 0:1024])
            nc.sync.dma_start(tv[:], tv_d[:])
            for k in range(KCH):
                ldq(k, 1)
            nc.gpsimd.dma_start(msk[:], msk_d[:])
            for k in range(KCH):
                ldp(k, 1)
                ldq(k, 2)
            for k in range(KCH):
                ldq(k, 3)

            se_sb = spool.tile([128, 2 * NU], f32, name="se_sb", tag="se_sb")
            cnt_sb = spool.tile([128, 2 * NU], f32, name="cnt_sb",
                                tag="cnt_sb")
            negc = spool.tile([128, 1], f32, name="negc", tag="negc")
            nc.vector.memset(negc[:], -C_SHIFT)

            for u in range(NU):
                nh, m = u // MT, u % MT
                ps = ppool.tile([128, 1024], f32, name="ps", tag="ps")
                for nloc in range(2):
                    nb = nh * 2 + nloc
                    for k in range(KCH):
                        nc.tensor.matmul(
                            ps[:, nloc * 512:(nloc + 1) * 512],
                            qk[k][:, m * 128:(m + 1) * 128],
                            pk[k][:, nb * 512:(nb + 1) * 512],
                            start=(k == 0),
                            stop=(k == KCH - 1),
                        )
                # per-bank stats: each 512-col bank's reducers fire as soon
                # as its 6-MM accumulation finishes, halving the PSUM
                # slot-release lag behind the PE.
                je = jpool.tile([128, 1024], bf16, name="je", tag="je")
                jc = jpool.tile([128, 1024], bf16, name="jc", tag="jc")
                for h in range(2):
                    sl = slice(h * 512, (h + 1) * 512)
                    col = 2 * u + h
                    nc.scalar.activation(
                        je[:, sl], ps[:, sl],
                        mybir.ActivationFunctionType.Exp,
                        bias=negc[:], scale=1.0,
                        accum_out=se_sb[:, col:col + 1],
                    )
                    if u in (OWN_M, MT + OWN_M + 1):
                        # half-tiles holding the self column: masked count
                        nc.vector.scalar_tensor_tensor(
                            out=jc[:, sl], in0=ps[:, sl],
                            scalar=tv[:, m:m + 1], in1=msk[:, sl],
                            op0=mybir.AluOpType.is_gt,
                            op1=mybir.AluOpType.mult,
                            accum_out=cnt_sb[:, col:col + 1],
                        )
                    else:
                        nc.vector.tensor_scalar(
                            jc[:, sl], ps[:, sl], tv[:, m:m + 1], None,
                            op0=mybir.AluOpType.is_gt,
                            op1=mybir.AluOpType.add,
                            accum_out=cnt_sb[:, col:col + 1],
                        )

            nc.sync.dma_start(se_d[:], se_sb[:])
            nc.gpsimd.dma_start(cnt_d[:], cnt_sb[:])

    nc.compile()
    return nc


def _np_dtype(mm_dt):
    if mm_dt == mybir.dt.bfloat16:
        import ml_dtypes
        return ml_dtypes.bfloat16
    return np.float32


def _perm(c):
    """Rotation putting core c's own queries at m-tiles OWN_M, OWN_M+1."""
    return np.roll(np.arange(B), OWN_M * 128 - c * QSLAB)


def prepare(q, p, mm_dt=None):
    """Host-side shard prep. Returns (in_maps, t32, perms)."""
    if mm_dt is None:
        mm_dt = MM_DT
    npdt = _np_dtype(mm_dt)
    q = np.ascontiguousarray(np.asarray(q, dtype=np.float32))
    p = np.ascontiguousarray(np.asarray(p, dtype=np.float32))

    # target scores t_i = q_i . p_{8i} (fp32; matches the reference's fp32
    # value to ~1e-7 — only a compare threshold + host-tail term)
    t32 = np.einsum("ij,ij->i", q, p[::NP], dtype=np.float64).astype(np.float32)

    qT = np.ascontiguousarray(q.T)  # [D, B] fp32
    r = np.arange(128)
    # self columns: unit 2*OWN_M has query pi=OWN_M*128+r vs local col 8r
    # (half 0); unit 2*OWN_M+3 has pi=(OWN_M+1)*128+r vs col 1024+8r
    # (i.e. col 8r of half 1). Same mask for both, same for every core.
    msk = np.ones((128, 1024), dtype=np.float32)
    msk[r, 8 * r] = 0.0

    in_maps = []
    perms = []
    for c in range(NCORES):
        perm = _perm(c)
        perms.append(perm)
        qTc = np.ascontiguousarray(qT[:, perm]).astype(npdt)
        pTc = np.ascontiguousarray(p[c * PSLAB:(c + 1) * PSLAB].T).astype(npdt)
        tvc = np.ascontiguousarray(t32[perm].reshape(MT, 128).T)
        in_maps.append({"qT": qTc, "pT": pTc, "tvec": tvc, "msk": msk})
    return in_maps, t32, perms


def finalize(results, t32, perms):
    """Combine per-core partials into the scalar loss (fp64 host tail)."""
    se_tot = np.zeros(B, dtype=np.float64)
    cnt_tot = np.zeros(B, dtype=np.float64)
    for c in range(NCORES):
        perm = perms[c]
        # column c = 32*nh + 2*m + h; query pi = m*128 + r
        se = results[c]["se_out"].astype(np.float64)
        cnt = results[c]["cnt_out"].astype(np.float64)
        se_q = se.reshape(128, 2, MT, 2).sum(axis=(1, 3)).T.ravel()
        cnt_q = cnt.reshape(128, 2, MT, 2).sum(axis=(1, 3)).T.ravel()
        se_tot[perm] += se_q
        cnt_tot[perm] += cnt_q
    lse = C_SHIFT + np.log(se_tot)
    raw = lse - t32.astype(np.float64)
    w = 1.0 + ALPHA * np.exp(-((cnt_tot - OPTIMAL_RANK) ** 2)
                             / (2.0 * SIGMA ** 2))
    return np.float32(np.mean(raw * w))


def _get_nc(mm_dt=None):
    if mm_dt is None:
        mm_dt = MM_DT
    if mm_dt not in _STATE:
        _STATE[mm_dt] = _build_nc(mm_dt)
    return _STATE[mm_dt]


def kernel(q_reps, p_reps, n_passages):
    assert int(np.asarray(n_passages)) == NP
    nc = _get_nc()
    in_maps, t32, perms = prepare(q_reps, p_reps)
    try:
        res = run_bass_kernel_spmd(nc, in_maps, core_ids=list(range(NCORES)))
    except Exception:
        # rare transient NRT_EXEC_UNIT_UNRECOVERABLE; reset the PJRT
        # client and retry once
        import time
        try:
            import jax
            jax.clear_caches()
            jax.extend.backend.clear_backends()
        except Exception:
            pass
        time.sleep(10)
        res = run_bass_kernel_spmd(nc, in_maps, core_ids=list(range(NCORES)))
    return finalize(res.results, t32, perms)


def run_profiled(q_reps, p_reps, n_passages, mm_dt=None, trace=True):
    """Same as kernel() but returns (loss, BassKernelResults) with NTFF
    profile (requires the antenv.axon_hooks shim; see _install_ntff_shim)."""
    nc = _get_nc(mm_dt)
    in_maps, t32, perms = prepare(q_reps, p_reps, mm_dt)
    res = run_bass_kernel_spmd(nc, in_maps, core_ids=list(range(NCORES)),
                               trace=trace)
    loss = finalize(res.results, t32, perms)
    return loss, res


def _install_ntff_shim():
    """Provide antenv.axon_hooks (absent in this image) so trace=True works."""
    import types
    import antenv
    if "antenv.axon_hooks" in sys.modules:
        return
    mod = types.ModuleType("antenv.axon_hooks")
    mod._hook = None
    mod.set_axon_ntff_profile_hook = lambda h: setattr(mod, "_hook", h)
    mod.get_axon_ntff_profile_hook = lambda: mod._hook
    sys.modules["antenv.axon_hooks"] = mod
    antenv.axon_hooks = mod
    try:
        from trn_agent_boot.trn_boot import _ntff_profile_via_ctypes
        hook = _ntff_profile_via_ctypes("/opt/axon/libaxon_pjrt.so")
        if hook is not None:
            mod._hook = hook
    except Exception:
        pass



# revision 3
# speedup vs baseline: 1.3305x; 1.3305x over previous
"""Trainium2 Bass kernel for nn_DenseModel_51926154609008 (weighted-rank
contrastive CE loss) — fp8 DoubleRow edition.

Math (reference semantics, no sort needed):
  scores = q @ p.T                       [B=2048, P=16384]
  t_i    = scores[i, 8*i]                (positive/target score, exact fp32
                                          on host)
  rank_i = #{j : scores[i, j] > t_i}     (argsort position == exceed count)
  lse_i  = logsumexp(scores[i, :])
  loss   = mean((lse_i - t_i) * (1 + 2.6*exp(-(rank_i-1)^2 / (2*1.8^2))))

Sharding: passage-parallel (P split across 8 cores, q replicated).

fp8 strategy: q, p quantized host-side to e4m3 (ml_dtypes.float8_e4m3).
PE runs MatmulPerfMode.DoubleRow (2 fp8 k-chunks of 128 per instruction,
0.5 cycles per moving row = 157 TF/s, 2x the bf16 rate), so each
[128q x 512p] PSUM bank takes 3 matmuls instead of 6. Score error std
~1.04 (vs std 27.7 scores); host-emulated loss rel err 3.4e-4, far
under the 2e-2 gate. Ranks only matter for rank<=~8 queries (Gaussian
weight dies by rank 10) whose top-score gaps are >> the fp8 noise.

Consumer restructure (the bf16 kernel's ACT/DVE chain would bottleneck
at ~2x the fp8 PE rate):
  - One ACTIVATE Exp per [128,1024] unit (not per 512-bank), writing
    je bf16 to SBUF.  For NACC of the 32 units the ACT accumulator
    also emits the per-query sumexp column directly.
  - The other units' sumexp comes from a DVE tensor_reduce over je
    (bf16 2-byte dtype -> DVE 2x perf mode eligibility).
  - Rank counts compare je > theta_i = exp(t_i - C) on DVE reading
    bf16 SBUF (exp is monotone; bf16 rounding flips only |s-t| <~ 2^-9
    which is noise vs the fp8 error).  Self-column masked via one
    scalar_tensor_tensor with a bf16 0/1 mask on the two units that
    contain own queries (data rotated so they land at m-tiles 8,9).
    Optionally NPOOL of the counts run on GpSimd to offload DVE.
  - theta underflow (t_i < ~40 -> theta ~ 0 in fp32) only mis-counts
    queries whose true rank is already in the hundreds+, where the
    Gaussian weight is exactly 1 either way.

Host combines per-unit partials ([128, 32] sumexp + count tiles per
core) and evaluates the tiny [2048] tail in fp64.
"""

import sys

import numpy as np

sys.path.insert(0, "/opt/trn_rl_repo")

import concourse.bacc as bacc  # noqa: E402
import concourse.bass as bass  # noqa: E402
import concourse.mybir as mybir  # noqa: E402
import concourse.tile as tile  # noqa: E402
from concourse.bass_utils import run_bass_kernel_spmd  # noqa: E402

# Problem shape (hardcoded per the task contract).
B = 2048
D = 768
NP = 8
P = B * NP  # 16384
NCORES = 8
PSLAB = P // NCORES  # 2048 passage columns per core
KCH = D // 128  # 6 contraction chunks
KP = KCH // 2  # 3 DoubleRow chunk-pairs
MT = B // 128  # 16 query m-tiles
NU = 2 * MT  # 32 units of [128, 1024]
QSLAB = B // NCORES  # 256 queries owned per core
OWN_M = 8  # own queries sit at m-tiles 8,9

C_SHIFT = 128.0  # fixed exp shift: exp(s - C) never overflows

ALPHA = 2.6
OPTIMAL_RANK = 1.0
SIGMA = 1.8

# Consumer load-balancing knobs (tuned against HW trace):
# units with _is_acc(u) use the ACT accumulator for sumexp; the rest get a
# DVE tensor_reduce over je.  Units in _POOL_CNT run their count on GpSimd.
NACC = 16
_POOL_CNT = frozenset()

_STATE: dict = {}


def _is_acc(u):
    return (u % (NU // NACC)) == 0 if NACC else False


def _build_nc():
    nc = bacc.Bacc("TRN2", target_bir_lowering=False, debug=False,
                   num_devices=NCORES)

    f32 = mybir.dt.float32
    bf16 = mybir.dt.bfloat16
    fp8 = mybir.dt.float8e4

    # DRAM layout: [KCH, 128, cols] so chunk k DMAs to tile[:, k, :].
    qT_d = nc.dram_tensor("qT", [KCH, 128, B], fp8, kind="ExternalInput").ap()
    pT_d = nc.dram_tensor("pT", [KCH, 128, PSLAB], fp8,
                          kind="ExternalInput").ap()
    th_d = nc.dram_tensor("thv", [128, MT], f32, kind="ExternalInput").ap()
    msk_d = nc.dram_tensor("msk", [128, 1024], bf16, kind="ExternalInput").ap()
    se_d = nc.dram_tensor("se_out", [128, NU], f32, kind="ExternalOutput").ap()
    cnt_d = nc.dram_tensor("cnt_out", [128, NU], f32,
                           kind="ExternalOutput").ap()

    with tile.TileContext(nc) as tc:
        with (
            tc.tile_pool(name="weights", bufs=1) as wpool,
            tc.tile_pool(name="stats", bufs=1) as spool,
            tc.tile_pool(name="je", bufs=3) as jepool,
            tc.tile_pool(name="jc", bufs=2) as jcpool,
            tc.tile_pool(name="psum", bufs=4,
                         space=bass.MemorySpace.PSUM) as ppool,
        ):
            q8 = wpool.tile([128, KCH, B], fp8, name="q8", tag="q8")
            p8 = wpool.tile([128, KCH, PSLAB], fp8, name="p8", tag="p8")

            def ldq(k, part):  # [128, 512] query col-chunks on GpSimd's seq
                nc.gpsimd.dma_start(
                    q8[:, k, part * 512:(part + 1) * 512],
                    qT_d[k, :, part * 512:(part + 1) * 512])

            def ldp(k, half):  # [128, 1024] passage half-chunks on Sync's seq
                nc.sync.dma_start(
                    p8[:, k, half * 1024:(half + 1) * 1024],
                    pT_d[k, :, half * 1024:(half + 1) * 1024])

            thv = spool.tile([128, MT], f32, name="thv", tag="thv")
            msk = spool.tile([128, 1024], bf16, name="msk", tag="msk")
            # smallest-possible first dependencies: MM#0 (DoubleRow over
            # k-chunks 0,1) needs q8 k0,k1 cols 0:128 and p8 k0,k1 cols
            # 0:512.  All transfers are plain 2-D [128, cols] chunks: mixed
            # (dest [128,2,N] <- src [2,128,N]) patterns scramble data.
            nc.gpsimd.dma_start(q8[:, 0, 0:128], qT_d[0, :, 0:128])
            nc.gpsimd.dma_start(q8[:, 1, 0:128], qT_d[1, :, 0:128])
            nc.sync.dma_start(p8[:, 0, 0:512], pT_d[0, :, 0:512])
            nc.sync.dma_start(p8[:, 1, 0:512], pT_d[1, :, 0:512])
            nc.gpsimd.dma_start(q8[:, 0, 128:512], qT_d[0, :, 128:512])
            nc.gpsimd.dma_start(q8[:, 1, 128:512], qT_d[1, :, 128:512])
            nc.sync.dma_start(p8[:, 0, 512:1024], pT_d[0, :, 512:1024])
            nc.sync.dma_start(p8[:, 1, 512:1024], pT_d[1, :, 512:1024])
            for k in range(2, KCH):
                ldq(k, 0)
                if k <= 3:
                    ldp(k, 0)
                else:
                    # third sequencer so the k-chain of first-unit operands
                    # arrives faster than the PE consumes it
                    nc.scalar.dma_start(p8[:, k, 0:1024], pT_d[k, :, 0:1024])
            nc.sync.dma_start(thv[:], th_d[:])
            for k in range(KCH):
                ldq(k, 1)
            nc.gpsimd.dma_start(msk[:], msk_d[:])
            for k in range(KCH):
                ldp(k, 1)
                ldq(k, 2)
            for k in range(KCH):
                ldq(k, 3)

            se_sb = spool.tile([128, NU], f32, name="se_sb", tag="se_sb")
            cnt_sb = spool.tile([128, NU], f32, name="cnt_sb", tag="cnt_sb")
            negc = spool.tile([128, 1], f32, name="negc", tag="negc")
            nc.vector.memset(negc[:], -C_SHIFT)

            dr = mybir.MatmulPerfMode.DoubleRow
            for u in range(NU):
                nh, m = u // MT, u % MT
                ps = ppool.tile([128, 1024], f32, name="ps", tag="ps")
                for nloc in range(2):
                    nb = nh * 2 + nloc
                    for c in range(KP):
                        nc.tensor.matmul(
                            ps[:, nloc * 512:(nloc + 1) * 512],
                            q8[:, 2 * c:2 * c + 2, m * 128:(m + 1) * 128],
                            p8[:, 2 * c:2 * c + 2, nb * 512:(nb + 1) * 512],
                            start=(c == 0),
                            stop=(c == KP - 1),
                            perf_mode=dr,
                        )
                je = jepool.tile([128, 1024], bf16, name="je", tag="je")
                jc = jcpool.tile([128, 1024], bf16, name="jc", tag="jc")
                if _is_acc(u):
                    nc.scalar.activation(
                        je[:], ps[:], mybir.ActivationFunctionType.Exp,
                        bias=negc[:], scale=1.0,
                        accum_out=se_sb[:, u:u + 1],
                    )
                else:
                    nc.scalar.activation(
                        je[:], ps[:], mybir.ActivationFunctionType.Exp,
                        bias=negc[:], scale=1.0,
                    )
                    nc.vector.tensor_reduce(
                        se_sb[:, u:u + 1], je[:],
                        axis=mybir.AxisListType.X, op=mybir.AluOpType.add,
                    )
                if u in (OWN_M, MT + OWN_M + 1):
                    # units holding the self column: masked count
                    nc.vector.scalar_tensor_tensor(
                        out=jc[:], in0=je[:],
                        scalar=thv[:, m:m + 1], in1=msk[:],
                        op0=mybir.AluOpType.is_gt,
                        op1=mybir.AluOpType.mult,
                        accum_out=cnt_sb[:, u:u + 1],
                    )
                else:
                    eng = nc.gpsimd if u in _POOL_CNT else nc.vector
                    eng.tensor_scalar(
                        jc[:], je[:], thv[:, m:m + 1], None,
                        op0=mybir.AluOpType.is_gt,
                        op1=mybir.AluOpType.add,
                        accum_out=cnt_sb[:, u:u + 1],
                    )

            nc.sync.dma_start(se_d[:], se_sb[:])
            nc.gpsimd.dma_start(cnt_d[:], cnt_sb[:])

    nc.compile()
    return nc


def _perm(c):
    """Rotation putting core c's own queries at m-tiles OWN_M, OWN_M+1."""
    return np.roll(np.arange(B), OWN_M * 128 - c * QSLAB)


def prepare(q, p):
    """Host-side shard prep. Returns (in_maps, t32, perms)."""
    import ml_dtypes
    fp8 = ml_dtypes.float8_e4m3
    q = np.ascontiguousarray(np.asarray(q, dtype=np.float32))
    p = np.ascontiguousarray(np.asarray(p, dtype=np.float32))

    # target scores t_i = q_i . p_{8i} (exact fp32; threshold + host tail)
    t32 = np.einsum("ij,ij->i", q, p[::NP], dtype=np.float64).astype(np.float32)
    # count threshold in exp space: theta_i = exp(t_i - C); underflow to 0
    # only affects queries whose rank is huge (weight exactly 1) either way
    th32 = np.exp(t32.astype(np.float64) - C_SHIFT).astype(np.float32)

    q8 = q.astype(fp8)  # [B, D]
    p8 = p.astype(fp8)  # [P, D]
    # DRAM layout [KCH, 128, cols]: chunk k holds rows k*128..k*128+127 of
    # the transposed [D, cols] operand
    qT8 = np.ascontiguousarray(q8.T.reshape(KCH, 128, B))

    r = np.arange(128)
    # self columns: unit (nh=0, m=OWN_M) row r vs local col 8r; unit
    # (nh=1, m=OWN_M+1) row r vs col 1024+8r of half 1 (same in-unit col 8r)
    msk = np.ones((128, 1024), dtype=ml_dtypes.bfloat16)
    msk[r, 8 * r] = 0.0

    in_maps = []
    perms = []
    for c in range(NCORES):
        perm = _perm(c)
        perms.append(perm)
        qTc = np.ascontiguousarray(qT8[:, :, perm])
        pTc = np.ascontiguousarray(
            p8[c * PSLAB:(c + 1) * PSLAB].T.reshape(KCH, 128, PSLAB))
        thc = np.ascontiguousarray(th32[perm].reshape(MT, 128).T)
        in_maps.append({"qT": qTc, "pT": pTc, "thv": thc, "msk": msk})
    return in_maps, t32, perms


def finalize(results, t32, perms):
    """Combine per-core partials into the scalar loss (fp64 host tail)."""
    se_tot = np.zeros(B, dtype=np.float64)
    cnt_tot = np.zeros(B, dtype=np.float64)
    for c in range(NCORES):
        perm = perms[c]
        # col u = nh*MT + m; query pi = m*128 + r
        se = results[c]["se_out"].astype(np.float64)
        cnt = results[c]["cnt_out"].astype(np.float64)
        se_q = se.reshape(128, 2, MT).sum(axis=1).T.ravel()
        cnt_q = cnt.reshape(128, 2, MT).sum(axis=1).T.ravel()
        se_tot[perm] += se_q
        cnt_tot[perm] += cnt_q
    lse = C_SHIFT + np.log(se_tot)
    raw = lse - t32.astype(np.float64)
    w = 1.0 + ALPHA * np.exp(-((cnt_tot - OPTIMAL_RANK) ** 2)
                             / (2.0 * SIGMA ** 2))
    return np.float32(np.mean(raw * w))


def _get_nc():
    if "nc" not in _STATE:
        _STATE["nc"] = _build_nc()
    return _STATE["nc"]


def kernel(q_reps, p_reps, n_passages):
    assert int(np.asarray(n_passages)) == NP
    nc = _get_nc()
    in_maps, t32, perms = prepare(q_reps, p_reps)
    try:
        res = run_bass_kernel_spmd(nc, in_maps, core_ids=list(range(NCORES)))
    except Exception:
        # rare transient NRT_EXEC_UNIT_UNRECOVERABLE; reset the PJRT
        # client and retry once
        import time
        try:
            import jax
            jax.clear_caches()
            jax.extend.backend.clear_backends()
        except Exception:
            pass
        time.sleep(10)
        res = run_bass_kernel_spmd(nc, in_maps, core_ids=list(range(NCORES)))
    return finalize(res.results, t32, perms)


def run_profiled(q_reps, p_reps, n_passages, trace=True):
    """Same as kernel() but returns (loss, BassKernelResults) with NTFF
    profile (requires the antenv.axon_hooks shim; see _install_ntff_shim)."""
    nc = _get_nc()
    in_maps, t32, perms = prepare(q_reps, p_reps)
    res = run_bass_kernel_spmd(nc, in_maps, core_ids=list(range(NCORES)),
                               trace=trace)
    loss = finalize(res.results, t32, perms)
    return loss, res


def _install_ntff_shim():
    """Provide antenv.axon_hooks (absent in this image) so trace=True works."""
    import types
    import antenv
    if "antenv.axon_hooks" in sys.modules:
        return
    mod = types.ModuleType("antenv.axon_hooks")
    mod._hook = None
    mod.set_axon_ntff_profile_hook = lambda h: setattr(mod, "_hook", h)
    mod.get_axon_ntff_profile_hook = lambda: mod._hook
    sys.modules["antenv.axon_hooks"] = mod
    antenv.axon_hooks = mod
    try:
        from trn_agent_boot.trn_boot import _ntff_profile_via_ctypes
        hook = _ntff_profile_via_ctypes("/opt/axon/libaxon_pjrt.so")
        if hook is not None:
            mod._hook = hook
    except Exception:
        pass


# revision 5
# speedup vs baseline: 1.4381x; 1.0809x over previous
"""Trainium2 Bass kernel for nn_DenseModel_51926154609008 (weighted-rank
contrastive CE loss) — fp8 DoubleRow edition.

Math (reference semantics, no sort needed):
  scores = q @ p.T                       [B=2048, P=16384]
  t_i    = scores[i, 8*i]                (positive/target score, exact fp32
                                          on host)
  rank_i = #{j : scores[i, j] > t_i}     (argsort position == exceed count)
  lse_i  = logsumexp(scores[i, :])
  loss   = mean((lse_i - t_i) * (1 + 2.6*exp(-(rank_i-1)^2 / (2*1.8^2))))

Sharding: passage-parallel (P split across 8 cores, q replicated).

fp8 strategy: q, p quantized host-side to e4m3 (ml_dtypes.float8_e4m3).
PE runs MatmulPerfMode.DoubleRow (2 fp8 k-chunks of 128 per instruction,
0.5 cycles per moving row = 157 TF/s, 2x the bf16 rate), so each
[128q x 512p] PSUM bank takes 3 matmuls instead of 6 (192 total, 216 ns
steady spacing measured).  Score error std ~1.04 (scores' std is 27.7);
host-emulated loss rel err 3.4e-4, far under the 2e-2 gate.  Ranks only
matter for rank<=~8 queries (the Gaussian weight dies by rank 10) whose
top-score gaps are >> the fp8 noise.

m-major consumer structure (one [128, 2048] 4-bank PSUM tile per query
m-tile, double buffered over the 8 banks): the PE fills a tile with 12
DoubleRow matmuls (2.59 us); one 2048-wide ACTIVATE Exp (2.2 us
including the fused accumulator read) produces the per-query slab
sumexp AND a bf16 junk exp tile je in SBUF; one 2048-wide DVE count
(2.3 us) compares je > theta_i = exp(t_i - C) (exp is monotone; bf16
rounding only flips |s - t| <~ 2^-9 which is noise vs the fp8 error).
Every engine's per-m-tile cost sits under the PE's 2.59 us, so the
kernel is PE-bound at the fp8 roofline.  PSUM banks are released by the
ACT alone (the count reads je from SBUF, not PSUM).

Self-column masking: inputs are rotated per-core so own queries land at
m-tiles 8, 9; those two counts use scalar_tensor_tensor with a bf16 0/1
mask ([128,2048], zero at (r, 8r) resp. (r, 1024+8r)).

theta underflow (t_i < ~40 -> theta ~ 0 in fp32/bf16) only mis-counts
queries whose true rank is already in the hundreds+, where the Gaussian
weight is exactly 1 either way.

DMA: issue cost is ~0.65 us per dma_start on the issuing sequencer, so
the first m-tile's operands are spread over four sequencers (Sync +
Scalar: p8, GpSimd: q8 first columns + masks, Vector: q8 bulk).

Host combines per-m-tile partials ([128, 16] sumexp + count tiles per
core) and evaluates the tiny [2048] tail in fp64.
"""

import sys

import numpy as np

sys.path.insert(0, "/opt/trn_rl_repo")

import concourse.bacc as bacc  # noqa: E402
import concourse.bass as bass  # noqa: E402
import concourse.mybir as mybir  # noqa: E402
import concourse.tile as tile  # noqa: E402
from concourse.bass_utils import run_bass_kernel_spmd  # noqa: E402

# Problem shape (hardcoded per the task contract).
B = 2048
D = 768
NP = 8
P = B * NP  # 16384
NCORES = 8
PSLAB = P // NCORES  # 2048 passage columns per core
KCH = D // 128  # 6 contraction chunks
KP = KCH // 2  # 3 DoubleRow chunk-pairs
MT = B // 128  # 16 query m-tiles
QSLAB = B // NCORES  # 256 queries owned per core
OWN_M = 8  # own queries sit at m-tiles 8,9

C_SHIFT = 128.0  # fixed exp shift: exp(s - C) never overflows

ALPHA = 2.6
OPTIMAL_RANK = 1.0
SIGMA = 1.8

_STATE: dict = {}


def _build_nc():
    nc = bacc.Bacc("TRN2", target_bir_lowering=False, debug=False,
                   num_devices=NCORES)

    f32 = mybir.dt.float32
    bf16 = mybir.dt.bfloat16
    fp8 = mybir.dt.float8e4

    # DRAM layout: [KCH, 128, cols] so chunk k DMAs to tile[:, k, :].
    qT_d = nc.dram_tensor("qT", [KCH, 128, B], fp8, kind="ExternalInput").ap()
    pT_d = nc.dram_tensor("pT", [KCH, 128, PSLAB], fp8,
                          kind="ExternalInput").ap()
    th_d = nc.dram_tensor("thv", [128, MT], f32, kind="ExternalInput").ap()
    msk_d = nc.dram_tensor("msk", [2, 128, PSLAB], bf16,
                           kind="ExternalInput").ap()
    se_d = nc.dram_tensor("se_out", [128, MT], f32, kind="ExternalOutput").ap()
    cnt_d = nc.dram_tensor("cnt_out", [128, MT], f32,
                           kind="ExternalOutput").ap()

    with tile.TileContext(nc) as tc:
        with (
            tc.tile_pool(name="weights", bufs=1) as wpool,
            tc.tile_pool(name="stats", bufs=1) as spool,
            tc.tile_pool(name="je", bufs=3) as jepool,
            tc.tile_pool(name="jc", bufs=2) as jcpool,
            tc.tile_pool(name="psum", bufs=2,
                         space=bass.MemorySpace.PSUM) as ppool,
        ):
            q8 = wpool.tile([128, KCH, B], fp8, name="q8", tag="q8")
            p8 = wpool.tile([128, KCH, PSLAB], fp8, name="p8", tag="p8")
            thv = spool.tile([128, MT], f32, name="thv", tag="thv")
            msk8 = spool.tile([128, PSLAB], bf16, name="msk8", tag="msk8")
            msk9 = spool.tile([128, PSLAB], bf16, name="msk9", tag="msk9")
            se_sb = spool.tile([128, MT], f32, name="se_sb", tag="se_sb")
            cnt_sb = spool.tile([128, MT], f32, name="cnt_sb", tag="cnt_sb")
            negc = spool.tile([128, 1], f32, name="negc", tag="negc")

            # --- input DMA schedule (all plain 2-D [128, cols] chunks;
            # mixed-rank patterns scramble data) -------------------------
            # m-tile 0 needs q8[:, k, 0:128] for all k and ALL of p8, so
            # p8 is split over two sequencers and q8's first columns go
            # ahead of the bulk on a third.
            nc.vector.memset(negc[:], -C_SHIFT)
            for k in range(KCH):
                nc.gpsimd.dma_start(q8[:, k, 0:128], qT_d[k, :, 0:128])
            for k in range(4):
                nc.sync.dma_start(p8[:, k, 0:1024], pT_d[k, :, 0:1024])
            nc.scalar.dma_start(p8[:, 4, 0:1024], pT_d[4, :, 0:1024])
            nc.scalar.dma_start(p8[:, 5, 0:1024], pT_d[5, :, 0:1024])
            for k in range(3):
                nc.gpsimd.dma_start(q8[:, k, 128:2048], qT_d[k, :, 128:2048])
            for k in range(3, KCH):
                nc.scalar.dma_start(q8[:, k, 128:2048], qT_d[k, :, 128:2048])
            for k in range(4):
                nc.sync.dma_start(p8[:, k, 1024:2048], pT_d[k, :, 1024:2048])
            nc.scalar.dma_start(p8[:, 4, 1024:2048], pT_d[4, :, 1024:2048])
            nc.scalar.dma_start(p8[:, 5, 1024:2048], pT_d[5, :, 1024:2048])
            nc.scalar.dma_start(thv[:], th_d[:])
            nc.gpsimd.dma_start(msk8[:], msk_d[0])
            nc.gpsimd.dma_start(msk9[:], msk_d[1])

            dr = mybir.MatmulPerfMode.DoubleRow
            for m in range(MT):
                ps = ppool.tile([128, PSLAB], f32, name="ps", tag="ps")
                for b in range(4):
                    for c in range(KP):
                        nc.tensor.matmul(
                            ps[:, b * 512:(b + 1) * 512],
                            q8[:, 2 * c:2 * c + 2, m * 128:(m + 1) * 128],
                            p8[:, 2 * c:2 * c + 2, b * 512:(b + 1) * 512],
                            start=(c == 0),
                            stop=(c == KP - 1),
                            perf_mode=dr,
                        )
                je = jepool.tile([128, PSLAB], bf16, name="je", tag="je")
                jc = jcpool.tile([128, PSLAB], bf16, name="jc", tag="jc")
                nc.scalar.activation(
                    je[:], ps[:], mybir.ActivationFunctionType.Exp,
                    bias=negc[:], scale=1.0,
                    accum_out=se_sb[:, m:m + 1],
                )
                if m in (OWN_M, OWN_M + 1):
                    nc.vector.scalar_tensor_tensor(
                        out=jc[:], in0=je[:],
                        scalar=thv[:, m:m + 1],
                        in1=(msk8 if m == OWN_M else msk9)[:],
                        op0=mybir.AluOpType.is_gt,
                        op1=mybir.AluOpType.mult,
                        accum_out=cnt_sb[:, m:m + 1],
                    )
                else:
                    nc.vector.tensor_scalar(
                        jc[:], je[:], thv[:, m:m + 1], None,
                        op0=mybir.AluOpType.is_gt,
                        op1=mybir.AluOpType.add,
                        accum_out=cnt_sb[:, m:m + 1],
                    )

            nc.sync.dma_start(se_d[:], se_sb[:])
            nc.gpsimd.dma_start(cnt_d[:], cnt_sb[:])

    nc.compile()
    return nc


def _perm(c):
    """Rotation putting core c's own queries at m-tiles OWN_M, OWN_M+1."""
    return np.roll(np.arange(B), OWN_M * 128 - c * QSLAB)


def prepare(q, p):
    """Host-side shard prep. Returns (in_maps, t32, perms)."""
    import ml_dtypes
    fp8 = ml_dtypes.float8_e4m3
    q = np.ascontiguousarray(np.asarray(q, dtype=np.float32))
    p = np.ascontiguousarray(np.asarray(p, dtype=np.float32))

    # target scores t_i = q_i . p_{8i} (exact fp32; threshold + host tail)
    t32 = np.einsum("ij,ij->i", q, p[::NP], dtype=np.float64).astype(np.float32)
    # count threshold in exp space: theta_i = exp(t_i - C); underflow to 0
    # only affects queries whose rank is huge (weight exactly 1) either way
    th32 = np.exp(t32.astype(np.float64) - C_SHIFT).astype(np.float32)

    q8 = q.astype(fp8)  # [B, D]
    p8 = p.astype(fp8)  # [P, D]
    # DRAM layout [KCH, 128, cols]: chunk k holds rows k*128..k*128+127 of
    # the transposed [D, cols] operand
    qT8 = np.ascontiguousarray(q8.T.reshape(KCH, 128, B))

    r = np.arange(128)
    # self columns: m-tile 8 row r vs local col 8r; m-tile 9 row r vs
    # col 1024 + 8r
    msk = np.ones((2, 128, PSLAB), dtype=ml_dtypes.bfloat16)
    msk[0, r, 8 * r] = 0.0
    msk[1, r, 1024 + 8 * r] = 0.0

    in_maps = []
    perms = []
    for c in range(NCORES):
        perm = _perm(c)
        perms.append(perm)
        qTc = np.ascontiguousarray(qT8[:, :, perm])
        pTc = np.ascontiguousarray(
            p8[c * PSLAB:(c + 1) * PSLAB].T.reshape(KCH, 128, PSLAB))
        thc = np.ascontiguousarray(th32[perm].reshape(MT, 128).T)
        in_maps.append({"qT": qTc, "pT": pTc, "thv": thc, "msk": msk})
    return in_maps, t32, perms


def finalize(results, t32, perms):
    """Combine per-core partials into the scalar loss (fp64 host tail)."""
    se_tot = np.zeros(B, dtype=np.float64)
    cnt_tot = np.zeros(B, dtype=np.float64)
    for c in range(NCORES):
        perm = perms[c]
        # col m, row r -> query pi = m*128 + r
        se = results[c]["se_out"].astype(np.float64)
        cnt = results[c]["cnt_out"].astype(np.float64)
        se_tot[perm] += se.T.ravel()
        cnt_tot[perm] += cnt.T.ravel()
    lse = C_SHIFT + np.log(se_tot)
    raw = lse - t32.astype(np.float64)
    w = 1.0 + ALPHA * np.exp(-((cnt_tot - OPTIMAL_RANK) ** 2)
                             / (2.0 * SIGMA ** 2))
    return np.float32(np.mean(raw * w))


def _get_nc():
    if "nc" not in _STATE:
        _STATE["nc"] = _build_nc()
    return _STATE["nc"]


def kernel(q_reps, p_reps, n_passages):
    assert int(np.asarray(n_passages)) == NP
    nc = _get_nc()
    in_maps, t32, perms = prepare(q_reps, p_reps)
    try:
        res = run_bass_kernel_spmd(nc, in_maps, core_ids=list(range(NCORES)))
    except Exception:
        # rare transient NRT_EXEC_UNIT_UNRECOVERABLE; reset the PJRT
        # client and retry once
        import time
        try:
            import jax
            jax.clear_caches()
            jax.extend.backend.clear_backends()
        except Exception:
            pass
        time.sleep(10)
        res = run_bass_kernel_spmd(nc, in_maps, core_ids=list(range(NCORES)))
    return finalize(res.results, t32, perms)


def run_profiled(q_reps, p_reps, n_passages, trace=True):
    """Same as kernel() but returns (loss, BassKernelResults) with NTFF
    profile (requires the antenv.axon_hooks shim; see _install_ntff_shim)."""
    nc = _get_nc()
    in_maps, t32, perms = prepare(q_reps, p_reps)
    res = run_bass_kernel_spmd(nc, in_maps, core_ids=list(range(NCORES)),
                               trace=trace)
    loss = finalize(res.results, t32, perms)
    return loss, res


def _install_ntff_shim():
    """Provide antenv.axon_hooks (absent in this image) so trace=True works."""
    import types
    import antenv
    if "antenv.axon_hooks" in sys.modules:
        return
    mod = types.ModuleType("antenv.axon_hooks")
    mod._hook = None
    mod.set_axon_ntff_profile_hook = lambda h: setattr(mod, "_hook", h)
    mod.get_axon_ntff_profile_hook = lambda: mod._hook
    sys.modules["antenv.axon_hooks"] = mod
    antenv.axon_hooks = mod
    try:
        from trn_agent_boot.trn_boot import _ntff_profile_via_ctypes
        hook = _ntff_profile_via_ctypes("/opt/axon/libaxon_pjrt.so")
        if hook is not None:
            mod._hook = hook
    except Exception:
        pass


# revision 10
# speedup vs baseline: 1.4677x; 1.0206x over previous
"""Trainium2 Bass kernel for nn_DenseModel_51926154609008 (weighted-rank
contrastive CE loss) — fp8 DoubleRow edition.

Math (reference semantics, no sort needed):
  scores = q @ p.T                       [B=2048, P=16384]
  t_i    = scores[i, 8*i]                (positive/target score, exact fp32
                                          on host)
  rank_i = #{j : scores[i, j] > t_i}     (argsort position == exceed count)
  lse_i  = logsumexp(scores[i, :])
  loss   = mean((lse_i - t_i) * (1 + 2.6*exp(-(rank_i-1)^2 / (2*1.8^2))))

Sharding: passage-parallel (P split across 8 cores, q replicated).

fp8 strategy: q, p quantized host-side to e4m3 (ml_dtypes.float8_e4m3).
PE runs MatmulPerfMode.DoubleRow (2 fp8 k-chunks of 128 per instruction,
0.5 cycles per moving row = 157 TF/s, 2x the bf16 rate), so each
[128q x 512p] PSUM bank takes 3 matmuls instead of 6 (192 total, 216 ns
steady spacing measured).  Score error std ~1.04 (scores' std is 27.7);
host-emulated loss rel err 3.4e-4, far under the 2e-2 gate.  Ranks only
matter for rank<=~8 queries (the Gaussian weight dies by rank 10) whose
top-score gaps are >> the fp8 noise.

m-major consumer structure (one [128, 2048] 4-bank PSUM tile per query
m-tile, double buffered over the 8 banks): the PE fills a tile with 12
DoubleRow matmuls (2.59 us); one 2048-wide ACTIVATE Exp (2.2 us
including the fused accumulator read) produces the per-query slab
sumexp AND a bf16 junk exp tile je in SBUF; one 2048-wide DVE count
(2.3 us) compares je > theta_i = exp(t_i - C) (exp is monotone; bf16
rounding only flips |s - t| <~ 2^-9 which is noise vs the fp8 error).
Every engine's per-m-tile cost sits under the PE's 2.59 us, so the
kernel is PE-bound at the fp8 roofline.  PSUM banks are released by the
ACT alone (the count reads je from SBUF, not PSUM).

Self-column masking: inputs are rotated per-core so own queries land at
m-tiles 8, 9; those two counts use scalar_tensor_tensor with a bf16 0/1
mask ([128,2048], zero at (r, 8r) resp. (r, 1024+8r)).

theta underflow (t_i < ~40 -> theta ~ 0 in fp32/bf16) only mis-counts
queries whose true rank is already in the hundreds+, where the Gaussian
weight is exactly 1 either way.

DMA: issue cost is ~0.65 us per dma_start on the issuing sequencer, so
the first m-tile's operands are spread over four sequencers (Sync +
Scalar: p8, GpSimd: q8 first columns + masks, Vector: q8 bulk).

Host combines per-m-tile partials ([128, 16] sumexp + count tiles per
core) and evaluates the tiny [2048] tail in fp64.
"""

import sys

import numpy as np

sys.path.insert(0, "/opt/trn_rl_repo")

import concourse.bacc as bacc  # noqa: E402
import concourse.bass as bass  # noqa: E402
import concourse.mybir as mybir  # noqa: E402
import concourse.tile as tile  # noqa: E402
from concourse.bass_utils import run_bass_kernel_spmd  # noqa: E402

# Problem shape (hardcoded per the task contract).
B = 2048
D = 768
NP = 8
P = B * NP  # 16384
NCORES = 8
PSLAB = P // NCORES  # 2048 passage columns per core
KCH = D // 128  # 6 contraction chunks
KP = KCH // 2  # 3 DoubleRow chunk-pairs
MT = B // 128  # 16 query m-tiles
QSLAB = B // NCORES  # 256 queries owned per core
OWN_M = 8  # own queries sit at m-tiles 8,9

C_SHIFT = 128.0  # fixed exp shift: exp(s - C) never overflows

ALPHA = 2.6
OPTIMAL_RANK = 1.0
SIGMA = 1.8

_STATE: dict = {}


def _build_nc():
    nc = bacc.Bacc("TRN2", target_bir_lowering=False, debug=False,
                   num_devices=NCORES)

    f32 = mybir.dt.float32
    bf16 = mybir.dt.bfloat16
    fp8 = mybir.dt.float8e4

    # DRAM layout: [KP, 2, 128, cols] so pair c plane kk DMAs to
    # q8p[c][:, kk, :] as a plain 2-D [128, cols] transfer.
    qT_d = nc.dram_tensor("qT", [KP, 2, 128, B], fp8,
                          kind="ExternalInput").ap()
    pT_d = nc.dram_tensor("pT", [KP, 2, 128, PSLAB], fp8,
                          kind="ExternalInput").ap()
    th_d = nc.dram_tensor("thv", [128, MT], f32, kind="ExternalInput").ap()
    se_d = nc.dram_tensor("se_out", [128, MT + 1], f32,
                          kind="ExternalOutput").ap()
    cnt_d = nc.dram_tensor("cnt_out", [128, MT + 1], f32,
                           kind="ExternalOutput").ap()

    with tile.TileContext(nc) as tc:
        with (
            tc.tile_pool(name="weights", bufs=1) as wpool,
            tc.tile_pool(name="stats", bufs=1) as spool,
            tc.tile_pool(name="je", bufs=3) as jepool,
            tc.tile_pool(name="jc", bufs=2) as jcpool,
            tc.tile_pool(name="psum", bufs=2,
                         space=bass.MemorySpace.PSUM) as ppool,
        ):
            # per-pair operand tiles: keeps the DoubleRow [128, 2, cols]
            # reads inside one tile so the scheduler's flat byte-range
            # dependency tracking doesn't chain them to unrelated loads
            q8p = [wpool.tile([128, 2, B], fp8, name=f"q8p{c}", tag=f"q8p{c}")
                   for c in range(KP)]
            p8p = [wpool.tile([128, 2, PSLAB], fp8, name=f"p8p{c}",
                              tag=f"p8p{c}") for c in range(KP)]
            thv = spool.tile([128, MT], f32, name="thv", tag="thv")
            msk8 = spool.tile([128, PSLAB], bf16, name="msk8", tag="msk8")
            msk9 = spool.tile([128, PSLAB], bf16, name="msk9", tag="msk9")
            it16 = spool.tile([128, PSLAB], mybir.dt.int16, name="it16",
                              tag="it16")
            se_sb = spool.tile([128, MT + 1], f32, name="se_sb", tag="se_sb")
            cnt_sb = spool.tile([128, MT + 1], f32, name="cnt_sb",
                                tag="cnt_sb")
            negc = spool.tile([128, 1], f32, name="negc", tag="negc")

            # --- input DMA schedule (all plain 2-D [128, cols] chunks;
            # mixed-rank patterns scramble data).  m-tile 0 needs all of
            # q8 cols 0:128 and ALL of p8, so p8 is pair-major across two
            # sequencers and q8 full planes go on a third.
            nc.vector.memset(negc[:], -C_SHIFT)
            for c in range(KP):
                nc.gpsimd.dma_start(q8p[c][:, 0, :], qT_d[c, 0])
                nc.gpsimd.dma_start(q8p[c][:, 1, :], qT_d[c, 1])
            for c in range(2):
                for kk in range(2):
                    for h in range(2):
                        nc.sync.dma_start(
                            p8p[c][:, kk, h * 1024:(h + 1) * 1024],
                            pT_d[c, kk, :, h * 1024:(h + 1) * 1024])
            for kk in range(2):
                for h in range(2):
                    nc.scalar.dma_start(
                        p8p[2][:, kk, h * 1024:(h + 1) * 1024],
                        pT_d[2, kk, :, h * 1024:(h + 1) * 1024])
            nc.scalar.dma_start(thv[:], th_d[:])
            # masks generated on-device (saves 1MB of ramp DMA traffic):
            # msk8[r, c] = (c - 8r != 0), msk9[r, c] = (c - 8r - 1024 != 0)
            nc.gpsimd.iota(it16[:], [[1, PSLAB]], base=0,
                           channel_multiplier=-8)
            nc.vector.tensor_scalar(msk8[:], it16[:], 0, None,
                                    op0=mybir.AluOpType.not_equal)
            nc.gpsimd.iota(it16[:], [[1, PSLAB]], base=-1024,
                           channel_multiplier=-8)
            nc.vector.tensor_scalar(msk9[:], it16[:], 0, None,
                                    op0=mybir.AluOpType.not_equal)

            dr = mybir.MatmulPerfMode.DoubleRow

            def consume(m, lo, hi, col):
                """Exp+sumexp (Scalar) and rank count (Vector) for
                ps[:, lo:hi] of m-tile m, accumulating into stats col."""
                sl = slice(lo, hi)
                nc.scalar.activation(
                    je[:, sl], ps[:, sl], mybir.ActivationFunctionType.Exp,
                    bias=negc[:], scale=1.0,
                    accum_out=se_sb[:, col:col + 1],
                )
                if m in (OWN_M, OWN_M + 1):
                    nc.vector.scalar_tensor_tensor(
                        out=jc[:, sl], in0=je[:, sl],
                        scalar=thv[:, m:m + 1],
                        in1=(msk8 if m == OWN_M else msk9)[:, sl],
                        op0=mybir.AluOpType.is_gt,
                        op1=mybir.AluOpType.mult,
                        accum_out=cnt_sb[:, col:col + 1],
                    )
                else:
                    nc.vector.tensor_scalar(
                        jc[:, sl], je[:, sl], thv[:, m:m + 1], None,
                        op0=mybir.AluOpType.is_gt,
                        op1=mybir.AluOpType.add,
                        accum_out=cnt_sb[:, col:col + 1],
                    )

            for m in range(MT):
                ps = ppool.tile([128, PSLAB], f32, name="ps", tag="ps")
                for b in range(4):
                    for c in range(KP):
                        nc.tensor.matmul(
                            ps[:, b * 512:(b + 1) * 512],
                            q8p[c][:, :, m * 128:(m + 1) * 128],
                            p8p[c][:, :, b * 512:(b + 1) * 512],
                            start=(c == 0),
                            stop=(c == KP - 1),
                            perf_mode=dr,
                        )
                je = jepool.tile([128, PSLAB], bf16, name="je", tag="je")
                jc = jcpool.tile([128, PSLAB], bf16, name="jc", tag="jc")
                if m == MT - 1:
                    # split the last tile's consumers so they overlap the
                    # final matmuls instead of serializing after them
                    consume(m, 0, 1024, m)
                    consume(m, 1024, 2048, m + 1)
                else:
                    consume(m, 0, 2048, m)

            nc.sync.dma_start(se_d[:], se_sb[:])
            nc.gpsimd.dma_start(cnt_d[:], cnt_sb[:])

    nc.compile()
    return nc


def _perm(c):
    """Rotation putting core c's own queries at m-tiles OWN_M, OWN_M+1."""
    return np.roll(np.arange(B), OWN_M * 128 - c * QSLAB)


def prepare(q, p):
    """Host-side shard prep. Returns (in_maps, t32, perms)."""
    import ml_dtypes
    fp8 = ml_dtypes.float8_e4m3
    q = np.ascontiguousarray(np.asarray(q, dtype=np.float32))
    p = np.ascontiguousarray(np.asarray(p, dtype=np.float32))

    # target scores t_i = q_i . p_{8i} (exact fp32; threshold + host tail)
    t32 = np.einsum("ij,ij->i", q, p[::NP], dtype=np.float64).astype(np.float32)
    # count threshold in exp space: theta_i = exp(t_i - C); underflow to 0
    # only affects queries whose rank is huge (weight exactly 1) either way
    th32 = np.exp(t32.astype(np.float64) - C_SHIFT).astype(np.float32)

    q8 = q.astype(fp8)  # [B, D]
    p8 = p.astype(fp8)  # [P, D]
    # DRAM layout [KP, 2, 128, cols]: pair c plane kk holds rows
    # (2c+kk)*128 .. +127 of the transposed [D, cols] operand
    qT8 = np.ascontiguousarray(q8.T.reshape(KP, 2, 128, B))

    in_maps = []
    perms = []
    for c in range(NCORES):
        perm = _perm(c)
        perms.append(perm)
        qTc = np.ascontiguousarray(qT8[:, :, :, perm])
        pTc = np.ascontiguousarray(
            p8[c * PSLAB:(c + 1) * PSLAB].T.reshape(KP, 2, 128, PSLAB))
        thc = np.ascontiguousarray(th32[perm].reshape(MT, 128).T)
        in_maps.append({"qT": qTc, "pT": pTc, "thv": thc})
    return in_maps, t32, perms


def finalize(results, t32, perms):
    """Combine per-core partials into the scalar loss (fp64 host tail)."""
    se_tot = np.zeros(B, dtype=np.float64)
    cnt_tot = np.zeros(B, dtype=np.float64)
    for c in range(NCORES):
        perm = perms[c]
        # col m, row r -> query pi = m*128 + r; the last m-tile's stats
        # are split across cols MT-1 and MT (half-slab each)
        se = results[c]["se_out"].astype(np.float64)
        cnt = results[c]["cnt_out"].astype(np.float64)
        se[:, MT - 1] += se[:, MT]
        cnt[:, MT - 1] += cnt[:, MT]
        se_tot[perm] += se[:, :MT].T.ravel()
        cnt_tot[perm] += cnt[:, :MT].T.ravel()
    lse = C_SHIFT + np.log(se_tot)
    raw = lse - t32.astype(np.float64)
    w = 1.0 + ALPHA * np.exp(-((cnt_tot - OPTIMAL_RANK) ** 2)
                             / (2.0 * SIGMA ** 2))
    return np.float32(np.mean(raw * w))


def _get_nc():
    if "nc" not in _STATE:
        _STATE["nc"] = _build_nc()
    return _STATE["nc"]


def kernel(q_reps, p_reps, n_passages):
    assert int(np.asarray(n_passages)) == NP
    nc = _get_nc()
    in_maps, t32, perms = prepare(q_reps, p_reps)
    try:
        res = run_bass_kernel_spmd(nc, in_maps, core_ids=list(range(NCORES)))
    except Exception:
        # rare transient NRT_EXEC_UNIT_UNRECOVERABLE; reset the PJRT
        # client and retry once
        import time
        try:
            import jax
            jax.clear_caches()
            jax.extend.backend.clear_backends()
        except Exception:
            pass
        time.sleep(10)
        res = run_bass_kernel_spmd(nc, in_maps, core_ids=list(range(NCORES)))
    return finalize(res.results, t32, perms)


def run_profiled(q_reps, p_reps, n_passages, trace=True):
    """Same as kernel() but returns (loss, BassKernelResults) with NTFF
    profile (requires the antenv.axon_hooks shim; see _install_ntff_shim)."""
    nc = _get_nc()
    in_maps, t32, perms = prepare(q_reps, p_reps)
    res = run_bass_kernel_spmd(nc, in_maps, core_ids=list(range(NCORES)),
                               trace=trace)
    loss = finalize(res.results, t32, perms)
    return loss, res


def _install_ntff_shim():
    """Provide antenv.axon_hooks (absent in this image) so trace=True works."""
    import types
    import antenv
    if "antenv.axon_hooks" in sys.modules:
        return
    mod = types.ModuleType("antenv.axon_hooks")
    mod._hook = None
    mod.set_axon_ntff_profile_hook = lambda h: setattr(mod, "_hook", h)
    mod.get_axon_ntff_profile_hook = lambda: mod._hook
    sys.modules["antenv.axon_hooks"] = mod
    antenv.axon_hooks = mod
    try:
        from trn_agent_boot.trn_boot import _ntff_profile_via_ctypes
        hook = _ntff_profile_via_ctypes("/opt/axon/libaxon_pjrt.so")
        if hook is not None:
            mod._hook = hook
    except Exception:
        pass


# revision 17
# speedup vs baseline: 1.5141x; 1.0316x over previous
"""Trainium2 Bass kernel for nn_DenseModel_51926154609008 (weighted-rank
contrastive CE loss) — fp8 DoubleRow edition.

Math (reference semantics, no sort needed):
  scores = q @ p.T                       [B=2048, P=16384]
  t_i    = scores[i, 8*i]                (positive/target score, exact fp32
                                          on host)
  rank_i = #{j : scores[i, j] > t_i}     (argsort position == exceed count)
  lse_i  = logsumexp(scores[i, :])
  loss   = mean((lse_i - t_i) * (1 + 2.6*exp(-(rank_i-1)^2 / (2*1.8^2))))

Sharding: passage-parallel (P split across 8 cores, q replicated).

fp8 strategy: q, p quantized host-side to e4m3 (ml_dtypes.float8_e4m3).
PE runs MatmulPerfMode.DoubleRow (2 fp8 k-chunks of 128 per instruction,
0.5 cycles per moving row = 157 TF/s, 2x the bf16 rate), so each
[128q x 512p] PSUM bank takes 3 matmuls instead of 6 (192 total, 216 ns
steady spacing measured).  Score error std ~1.04 (scores' std is 27.7);
host-emulated loss rel err 3.4e-4, far under the 2e-2 gate.  Ranks only
matter for rank<=~8 queries (the Gaussian weight dies by rank 10) whose
top-score gaps are >> the fp8 noise.

m-major consumer structure (one [128, 2048] 4-bank PSUM tile per query
m-tile, double buffered over the 8 banks): the PE fills a tile with 12
DoubleRow matmuls (2.59 us); one 2048-wide ACTIVATE Exp (2.2 us
including the fused accumulator read) produces the per-query slab
sumexp AND a bf16 junk exp tile je in SBUF; one 2048-wide DVE count
(2.3 us) compares je > theta_i = exp(t_i - C) (exp is monotone; bf16
rounding only flips |s - t| <~ 2^-9 which is noise vs the fp8 error).
Every engine's per-m-tile cost sits under the PE's 2.59 us, so the
kernel is PE-bound at the fp8 roofline.  PSUM banks are released by the
ACT alone (the count reads je from SBUF, not PSUM).

Self-column masking: inputs are rotated per-core so own queries land at
m-tiles 8, 9; those two counts use scalar_tensor_tensor with a bf16 0/1
mask ([128,2048], zero at (r, 8r) resp. (r, 1024+8r)).

theta underflow (t_i < ~40 -> theta ~ 0 in fp32/bf16) only mis-counts
queries whose true rank is already in the hundreds+, where the Gaussian
weight is exactly 1 either way.

DMA: issue cost is ~0.65 us per dma_start on the issuing sequencer, so
the first m-tile's operands are spread over four sequencers (Sync +
Scalar: p8, GpSimd: q8 first columns + masks, Vector: q8 bulk).

Host combines per-m-tile partials ([128, 16] sumexp + count tiles per
core) and evaluates the tiny [2048] tail in fp64.
"""

import sys

import numpy as np

sys.path.insert(0, "/opt/trn_rl_repo")

import concourse.bacc as bacc  # noqa: E402
import concourse.bass as bass  # noqa: E402
import concourse.mybir as mybir  # noqa: E402
import concourse.tile as tile  # noqa: E402
from concourse.bass_utils import run_bass_kernel_spmd  # noqa: E402

# Problem shape (hardcoded per the task contract).
B = 2048
D = 768
NP = 8
P = B * NP  # 16384
NCORES = 8
PSLAB = P // NCORES  # 2048 passage columns per core
KCH = D // 128  # 6 contraction chunks
KP = KCH // 2  # 3 DoubleRow chunk-pairs
MT = B // 128  # 16 query m-tiles
QSLAB = B // NCORES  # 256 queries owned per core
OWN_M = 8  # own queries sit at m-tiles 8,9

C_SHIFT = 128.0  # fixed exp shift: exp(s - C) never overflows

ALPHA = 2.6
OPTIMAL_RANK = 1.0
SIGMA = 1.8

_STATE: dict = {}


def _build_nc():
    nc = bacc.Bacc("TRN2", target_bir_lowering=False, debug=False,
                   num_devices=NCORES)

    f32 = mybir.dt.float32
    bf16 = mybir.dt.bfloat16
    fp8 = mybir.dt.float8e4

    # DRAM layout: [KP, 2, 128, cols] so pair c plane kk DMAs to
    # q8p[c][:, kk, :] as a plain 2-D [128, cols] transfer.
    qT_d = nc.dram_tensor("qT", [KP, 2, 128, B], fp8,
                          kind="ExternalInput").ap()
    pT_d = nc.dram_tensor("pT", [KP, 2, 128, PSLAB], fp8,
                          kind="ExternalInput").ap()
    th_d = nc.dram_tensor("thv", [128, MT], f32, kind="ExternalInput").ap()
    # stats output: cols 0:MT+1 = per-m-tile sumexp, MT+1:2MT+2 = counts
    st_d = nc.dram_tensor("st_out", [128, 2 * MT + 2], f32,
                          kind="ExternalOutput").ap()

    with tile.TileContext(nc) as tc:
        with (
            tc.tile_pool(name="weights", bufs=1) as wpool,
            tc.tile_pool(name="stats", bufs=1) as spool,
            tc.tile_pool(name="je", bufs=3) as jepool,
            tc.tile_pool(name="jc", bufs=2) as jcpool,
            tc.tile_pool(name="psum", bufs=2,
                         space=bass.MemorySpace.PSUM) as ppool,
        ):
            # per-pair operand tiles: keeps the DoubleRow [128, 2, cols]
            # reads inside one tile so the scheduler's flat byte-range
            # dependency tracking doesn't chain them to unrelated loads
            q8p = [wpool.tile([128, 2, B], fp8, name=f"q8p{c}", tag=f"q8p{c}")
                   for c in range(KP)]
            p8p = [wpool.tile([128, 2, PSLAB], fp8, name=f"p8p{c}",
                              tag=f"p8p{c}") for c in range(KP)]
            thv = spool.tile([128, MT], f32, name="thv", tag="thv")
            # one wide mask W[r, c] = (c - 8r - 1024 != 0), so
            # msk9 = W[:, 0:2048] (zero at 1024+8r) and
            # msk8 = W[:, 1024:3072] (zero at 8r within the slice)
            mskw = spool.tile([128, PSLAB + 1024], bf16, name="mskw",
                              tag="mskw")
            it16 = spool.tile([128, PSLAB + 1024], mybir.dt.int16,
                              name="it16", tag="it16")
            st_sb = spool.tile([128, 2 * MT + 2], f32, name="st_sb",
                               tag="st_sb")
            negc = spool.tile([128, 1], f32, name="negc", tag="negc")

            # --- input DMA schedule (all plain 2-D [128, cols] full-plane
            # chunks; mixed-rank patterns scramble data).  m-tile 0 needs
            # all of q8 cols 0:128 and ALL of p8, so p8 pairs 0-1 go on
            # Sync, pair 2 on Scalar, q8 planes on GpSimd.
            nc.vector.memset(negc[:], -C_SHIFT)
            for c in range(KP):
                nc.gpsimd.dma_start(q8p[c][:, 0, :], qT_d[c, 0])
                nc.gpsimd.dma_start(q8p[c][:, 1, :], qT_d[c, 1])
            for c in range(2):
                for kk in range(2):
                    nc.sync.dma_start(p8p[c][:, kk, :], pT_d[c, kk])
            nc.scalar.dma_start(p8p[2][:, 0, :], pT_d[2, 0])
            nc.scalar.dma_start(p8p[2][:, 1, :], pT_d[2, 1])
            nc.scalar.dma_start(thv[:], th_d[:])
            # masks generated on-device (saves 1MB of ramp DMA traffic)
            nc.gpsimd.iota(it16[:], [[1, PSLAB + 1024]], base=-1024,
                           channel_multiplier=-8)
            nc.vector.tensor_scalar(mskw[:], it16[:], 0, None,
                                    op0=mybir.AluOpType.not_equal)

            dr = mybir.MatmulPerfMode.DoubleRow

            def consume(m, lo, hi, col):
                """Exp+sumexp (Scalar) and rank count (Vector) for
                ps[:, lo:hi] of m-tile m, accumulating into stats col."""
                sl = slice(lo, hi)
                nc.scalar.activation(
                    je[:, sl], ps[:, sl], mybir.ActivationFunctionType.Exp,
                    bias=negc[:], scale=1.0,
                    accum_out=st_sb[:, col:col + 1],
                )
                ccol = MT + 1 + col
                if m in (OWN_M, OWN_M + 1):
                    off = 1024 if m == OWN_M else 0
                    nc.vector.scalar_tensor_tensor(
                        out=jc[:, sl], in0=je[:, sl],
                        scalar=thv[:, m:m + 1],
                        in1=mskw[:, off + lo:off + hi],
                        op0=mybir.AluOpType.is_gt,
                        op1=mybir.AluOpType.mult,
                        accum_out=st_sb[:, ccol:ccol + 1],
                    )
                else:
                    nc.vector.tensor_scalar(
                        jc[:, sl], je[:, sl], thv[:, m:m + 1], None,
                        op0=mybir.AluOpType.is_gt,
                        op1=mybir.AluOpType.add,
                        accum_out=st_sb[:, ccol:ccol + 1],
                    )

            for m in range(MT):
                ps = ppool.tile([128, PSLAB], f32, name="ps", tag="ps")
                for b in range(4):
                    for c in range(KP):
                        nc.tensor.matmul(
                            ps[:, b * 512:(b + 1) * 512],
                            q8p[c][:, :, m * 128:(m + 1) * 128],
                            p8p[c][:, :, b * 512:(b + 1) * 512],
                            start=(c == 0),
                            stop=(c == KP - 1),
                            perf_mode=dr,
                        )
                je = jepool.tile([128, PSLAB], bf16, name="je", tag="je")
                jc = jcpool.tile([128, PSLAB], bf16, name="jc", tag="jc")
                if m == MT - 1:
                    # split the last tile's consumers so they overlap the
                    # final matmuls instead of serializing after them
                    consume(m, 0, 1024, m)
                    consume(m, 1024, 2048, m + 1)
                else:
                    consume(m, 0, 2048, m)

            nc.sync.dma_start(st_d[:], st_sb[:])

    nc.compile()
    return nc


def _perm(c):
    """Rotation putting core c's own queries at m-tiles OWN_M, OWN_M+1."""
    return np.roll(np.arange(B), OWN_M * 128 - c * QSLAB)


def prepare(q, p):
    """Host-side shard prep. Returns (in_maps, t32, perms)."""
    import ml_dtypes
    fp8 = ml_dtypes.float8_e4m3
    q = np.ascontiguousarray(np.asarray(q, dtype=np.float32))
    p = np.ascontiguousarray(np.asarray(p, dtype=np.float32))

    # target scores t_i = q_i . p_{8i} (exact fp32; threshold + host tail)
    t32 = np.einsum("ij,ij->i", q, p[::NP], dtype=np.float64).astype(np.float32)
    # count threshold in exp space: theta_i = exp(t_i - C); underflow to 0
    # only affects queries whose rank is huge (weight exactly 1) either way
    th32 = np.exp(t32.astype(np.float64) - C_SHIFT).astype(np.float32)

    q8 = q.astype(fp8)  # [B, D]
    p8 = p.astype(fp8)  # [P, D]
    # DRAM layout [KP, 2, 128, cols]: pair c plane kk holds rows
    # (2c+kk)*128 .. +127 of the transposed [D, cols] operand
    qT8 = np.ascontiguousarray(q8.T.reshape(KP, 2, 128, B))

    in_maps = []
    perms = []
    for c in range(NCORES):
        perm = _perm(c)
        perms.append(perm)
        qTc = np.ascontiguousarray(qT8[:, :, :, perm])
        pTc = np.ascontiguousarray(
            p8[c * PSLAB:(c + 1) * PSLAB].T.reshape(KP, 2, 128, PSLAB))
        thc = np.ascontiguousarray(th32[perm].reshape(MT, 128).T)
        in_maps.append({"qT": qTc, "pT": pTc, "thv": thc})
    return in_maps, t32, perms


def finalize(results, t32, perms):
    """Combine per-core partials into the scalar loss (fp64 host tail)."""
    se_tot = np.zeros(B, dtype=np.float64)
    cnt_tot = np.zeros(B, dtype=np.float64)
    for c in range(NCORES):
        perm = perms[c]
        # col m, row r -> query pi = m*128 + r; the last m-tile's stats
        # are split across cols MT-1 and MT (half-slab each)
        st = results[c]["st_out"].astype(np.float64)
        se, cnt = st[:, :MT + 1], st[:, MT + 1:]
        se[:, MT - 1] += se[:, MT]
        cnt[:, MT - 1] += cnt[:, MT]
        se_tot[perm] += se[:, :MT].T.ravel()
        cnt_tot[perm] += cnt[:, :MT].T.ravel()
    lse = C_SHIFT + np.log(se_tot)
    raw = lse - t32.astype(np.float64)
    w = 1.0 + ALPHA * np.exp(-((cnt_tot - OPTIMAL_RANK) ** 2)
                             / (2.0 * SIGMA ** 2))
    return np.float32(np.mean(raw * w))


def _get_nc():
    if "nc" not in _STATE:
        _STATE["nc"] = _build_nc()
    return _STATE["nc"]


def kernel(q_reps, p_reps, n_passages):
    assert int(np.asarray(n_passages)) == NP
    nc = _get_nc()
    in_maps, t32, perms = prepare(q_reps, p_reps)
    try:
        res = run_bass_kernel_spmd(nc, in_maps, core_ids=list(range(NCORES)))
    except Exception:
        # rare transient NRT_EXEC_UNIT_UNRECOVERABLE; reset the PJRT
        # client and retry once
        import time
        try:
            import jax
            jax.clear_caches()
            jax.extend.backend.clear_backends()
        except Exception:
            pass
        time.sleep(10)
        res = run_bass_kernel_spmd(nc, in_maps, core_ids=list(range(NCORES)))
    return finalize(res.results, t32, perms)


def run_profiled(q_reps, p_reps, n_passages, trace=True):
    """Same as kernel() but returns (loss, BassKernelResults) with NTFF
    profile (requires the antenv.axon_hooks shim; see _install_ntff_shim)."""
    nc = _get_nc()
    in_maps, t32, perms = prepare(q_reps, p_reps)
    res = run_bass_kernel_spmd(nc, in_maps, core_ids=list(range(NCORES)),
                               trace=trace)
    loss = finalize(res.results, t32, perms)
    return loss, res


def _install_ntff_shim():
    """Provide antenv.axon_hooks (absent in this image) so trace=True works."""
    import types
    import antenv
    if "antenv.axon_hooks" in sys.modules:
        return
    mod = types.ModuleType("antenv.axon_hooks")
    mod._hook = None
    mod.set_axon_ntff_profile_hook = lambda h: setattr(mod, "_hook", h)
    mod.get_axon_ntff_profile_hook = lambda: mod._hook
    sys.modules["antenv.axon_hooks"] = mod
    antenv.axon_hooks = mod
    try:
        from trn_agent_boot.trn_boot import _ntff_profile_via_ctypes
        hook = _ntff_profile_via_ctypes("/opt/axon/libaxon_pjrt.so")
        if hook is not None:
            mod._hook = hook
    except Exception:
        pass


# revision 18
# speedup vs baseline: 1.5219x; 1.0051x over previous
"""Trainium2 Bass kernel for nn_DenseModel_51926154609008 (weighted-rank
contrastive CE loss) — fp8 DoubleRow edition.

Math (reference semantics, no sort needed):
  scores = q @ p.T                       [B=2048, P=16384]
  t_i    = scores[i, 8*i]                (positive/target score, exact fp32
                                          on host)
  rank_i = #{j : scores[i, j] > t_i}     (argsort position == exceed count)
  lse_i  = logsumexp(scores[i, :])
  loss   = mean((lse_i - t_i) * (1 + 2.6*exp(-(rank_i-1)^2 / (2*1.8^2))))

Sharding: passage-parallel (P split across 8 cores, q replicated).

fp8 strategy: q, p quantized host-side to e4m3 (ml_dtypes.float8_e4m3).
PE runs MatmulPerfMode.DoubleRow (2 fp8 k-chunks of 128 per instruction,
0.5 cycles per moving row = 157 TF/s, 2x the bf16 rate), so each
[128q x 512p] PSUM bank takes 3 matmuls instead of 6 (192 total, 216 ns
steady spacing measured).  Score error std ~1.04 (scores' std is 27.7);
host-emulated loss rel err 3.4e-4, far under the 2e-2 gate.  Ranks only
matter for rank<=~8 queries (the Gaussian weight dies by rank 10) whose
top-score gaps are >> the fp8 noise.

m-major consumer structure (one [128, 2048] 4-bank PSUM tile per query
m-tile, double buffered over the 8 banks): the PE fills a tile with 12
DoubleRow matmuls (2.59 us); one 2048-wide ACTIVATE Exp (2.2 us
including the fused accumulator read) produces the per-query slab
sumexp AND a bf16 junk exp tile je in SBUF; one 2048-wide DVE count
(2.3 us) compares je > theta_i = exp(t_i - C) (exp is monotone; bf16
rounding only flips |s - t| <~ 2^-9 which is noise vs the fp8 error).
Every engine's per-m-tile cost sits under the PE's 2.59 us, so the
kernel is PE-bound at the fp8 roofline.  PSUM banks are released by the
ACT alone (the count reads je from SBUF, not PSUM).

Self-column masking: inputs are rotated per-core so own queries land at
m-tiles 8, 9; those two counts use scalar_tensor_tensor with a bf16 0/1
mask ([128,2048], zero at (r, 8r) resp. (r, 1024+8r)).

theta underflow (t_i < ~40 -> theta ~ 0 in fp32/bf16) only mis-counts
queries whose true rank is already in the hundreds+, where the Gaussian
weight is exactly 1 either way.

DMA: issue cost is ~0.65 us per dma_start on the issuing sequencer, so
the first m-tile's operands are spread over four sequencers (Sync +
Scalar: p8, GpSimd: q8 first columns + masks, Vector: q8 bulk).

Host combines per-m-tile partials ([128, 16] sumexp + count tiles per
core) and evaluates the tiny [2048] tail in fp64.
"""

import sys

import numpy as np

sys.path.insert(0, "/opt/trn_rl_repo")

import concourse.bacc as bacc  # noqa: E402
import concourse.bass as bass  # noqa: E402
import concourse.mybir as mybir  # noqa: E402
import concourse.tile as tile  # noqa: E402
from concourse.bass_utils import run_bass_kernel_spmd  # noqa: E402

# Problem shape (hardcoded per the task contract).
B = 2048
D = 768
NP = 8
P = B * NP  # 16384
NCORES = 8
PSLAB = P // NCORES  # 2048 passage columns per core
KCH = D // 128  # 6 contraction chunks
KP = KCH // 2  # 3 DoubleRow chunk-pairs
MT = B // 128  # 16 query m-tiles
QSLAB = B // NCORES  # 256 queries owned per core
OWN_M = 8  # own queries sit at m-tiles 8,9

C_SHIFT = 128.0  # fixed exp shift: exp(s - C) never overflows

ALPHA = 2.6
OPTIMAL_RANK = 1.0
SIGMA = 1.8

_STATE: dict = {}


def _build_nc():
    nc = bacc.Bacc("TRN2", target_bir_lowering=False, debug=False,
                   num_devices=NCORES)

    f32 = mybir.dt.float32
    bf16 = mybir.dt.bfloat16
    fp8 = mybir.dt.float8e4

    # DRAM layout: [KP, 2, 128, cols] so pair c plane kk DMAs to
    # q8p[c][:, kk, :] as a plain 2-D [128, cols] transfer.
    qT_d = nc.dram_tensor("qT", [KP, 2, 128, B], fp8,
                          kind="ExternalInput").ap()
    pT_d = nc.dram_tensor("pT", [KP, 2, 128, PSLAB], fp8,
                          kind="ExternalInput").ap()
    th_d = nc.dram_tensor("thv", [128, MT], f32, kind="ExternalInput").ap()
    # stats output: cols 0:MT+1 = per-m-tile sumexp, MT+1:2MT+2 = counts
    st_d = nc.dram_tensor("st_out", [128, 2 * MT + 2], f32,
                          kind="ExternalOutput").ap()

    with tile.TileContext(nc) as tc:
        with (
            tc.tile_pool(name="weights", bufs=1) as wpool,
            tc.tile_pool(name="stats", bufs=1) as spool,
            tc.tile_pool(name="je", bufs=3) as jepool,
            tc.tile_pool(name="jc", bufs=2) as jcpool,
            tc.tile_pool(name="psum", bufs=2,
                         space=bass.MemorySpace.PSUM) as ppool,
        ):
            # per-pair operand tiles: keeps the DoubleRow [128, 2, cols]
            # reads inside one tile so the scheduler's flat byte-range
            # dependency tracking doesn't chain them to unrelated loads
            q8p = [wpool.tile([128, 2, B], fp8, name=f"q8p{c}", tag=f"q8p{c}")
                   for c in range(KP)]
            p8p = [wpool.tile([128, 2, PSLAB], fp8, name=f"p8p{c}",
                              tag=f"p8p{c}") for c in range(KP)]
            thv = spool.tile([128, MT], f32, name="thv", tag="thv")
            # one wide mask W[r, c] = (c - 8r - 1024 != 0), so
            # msk9 = W[:, 0:2048] (zero at 1024+8r) and
            # msk8 = W[:, 1024:3072] (zero at 8r within the slice)
            mskw = spool.tile([128, PSLAB + 1024], bf16, name="mskw",
                              tag="mskw")
            it16 = spool.tile([128, PSLAB + 1024], mybir.dt.int16,
                              name="it16", tag="it16")
            st_sb = spool.tile([128, 2 * MT + 2], f32, name="st_sb",
                               tag="st_sb")
            negc = spool.tile([128, 1], f32, name="negc", tag="negc")

            # --- input DMA schedule (all plain 2-D [128, cols] full-plane
            # chunks; mixed-rank patterns scramble data).  m-tile 0 needs
            # all of q8 cols 0:128 and ALL of p8, so p8 pairs 0-1 go on
            # Sync, pair 2 on Scalar, q8 planes on GpSimd.
            nc.vector.memset(negc[:], -C_SHIFT)
            for c in range(KP):
                nc.gpsimd.dma_start(q8p[c][:, 0, :], qT_d[c, 0])
                nc.gpsimd.dma_start(q8p[c][:, 1, :], qT_d[c, 1])
            for c in range(2):
                for kk in range(2):
                    nc.sync.dma_start(p8p[c][:, kk, :], pT_d[c, kk])
            nc.scalar.dma_start(p8p[2][:, 0, :], pT_d[2, 0])
            nc.scalar.dma_start(p8p[2][:, 1, :], pT_d[2, 1])
            nc.scalar.dma_start(thv[:], th_d[:])
            # masks generated on-device (saves 1MB of ramp DMA traffic)
            nc.gpsimd.iota(it16[:], [[1, PSLAB + 1024]], base=-1024,
                           channel_multiplier=-8)
            nc.vector.tensor_scalar(mskw[:], it16[:], 0, None,
                                    op0=mybir.AluOpType.not_equal)

            dr = mybir.MatmulPerfMode.DoubleRow

            def consume(m, lo, hi, col):
                """Exp+sumexp (Scalar) and rank count (Vector) for
                ps[:, lo:hi] of m-tile m, accumulating into stats col."""
                sl = slice(lo, hi)
                nc.scalar.activation(
                    je[:, sl], ps[:, sl], mybir.ActivationFunctionType.Exp,
                    bias=negc[:], scale=1.0,
                    accum_out=st_sb[:, col:col + 1],
                )
                ccol = MT + 1 + col
                if m in (OWN_M, OWN_M + 1):
                    off = 1024 if m == OWN_M else 0
                    nc.vector.scalar_tensor_tensor(
                        out=jc[:, sl], in0=je[:, sl],
                        scalar=thv[:, m:m + 1],
                        in1=mskw[:, off + lo:off + hi],
                        op0=mybir.AluOpType.is_gt,
                        op1=mybir.AluOpType.mult,
                        accum_out=st_sb[:, ccol:ccol + 1],
                    )
                else:
                    nc.vector.tensor_scalar(
                        jc[:, sl], je[:, sl], thv[:, m:m + 1], None,
                        op0=mybir.AluOpType.is_gt,
                        op1=mybir.AluOpType.add,
                        accum_out=st_sb[:, ccol:ccol + 1],
                    )

            def mm(ps, m, b, c):
                nc.tensor.matmul(
                    ps[:, b * 512:(b + 1) * 512],
                    q8p[c][:, :, m * 128:(m + 1) * 128],
                    p8p[c][:, :, b * 512:(b + 1) * 512],
                    start=(c == 0),
                    stop=(c == KP - 1),
                    perf_mode=dr,
                )

            # m-tiles 0,1 run pair-major (c outer) so the PE starts as soon
            # as operand pair 0 lands and overlaps the rest of the input
            # DMA (and its own p-state warmup) with real work.
            ps01 = [ppool.tile([128, PSLAB], f32, name="ps", tag="ps")
                    for _ in range(2)]
            for c in range(KP):
                for m in range(2):
                    for b in range(4):
                        mm(ps01[m], m, b, c)
            for m in range(2):
                ps = ps01[m]
                je = jepool.tile([128, PSLAB], bf16, name="je", tag="je")
                jc = jcpool.tile([128, PSLAB], bf16, name="jc", tag="jc")
                consume(m, 0, 2048, m)

            for m in range(2, MT):
                ps = ppool.tile([128, PSLAB], f32, name="ps", tag="ps")
                for b in range(4):
                    for c in range(KP):
                        mm(ps, m, b, c)
                je = jepool.tile([128, PSLAB], bf16, name="je", tag="je")
                jc = jcpool.tile([128, PSLAB], bf16, name="jc", tag="jc")
                if m == MT - 1:
                    # split the last tile's consumers so they overlap the
                    # final matmuls instead of serializing after them
                    consume(m, 0, 1024, m)
                    consume(m, 1024, 2048, m + 1)
                else:
                    consume(m, 0, 2048, m)

            nc.sync.dma_start(st_d[:], st_sb[:])

    nc.compile()
    return nc


def _perm(c):
    """Rotation putting core c's own queries at m-tiles OWN_M, OWN_M+1."""
    return np.roll(np.arange(B), OWN_M * 128 - c * QSLAB)


def prepare(q, p):
    """Host-side shard prep. Returns (in_maps, t32, perms)."""
    import ml_dtypes
    fp8 = ml_dtypes.float8_e4m3
    q = np.ascontiguousarray(np.asarray(q, dtype=np.float32))
    p = np.ascontiguousarray(np.asarray(p, dtype=np.float32))

    # target scores t_i = q_i . p_{8i} (exact fp32; threshold + host tail)
    t32 = np.einsum("ij,ij->i", q, p[::NP], dtype=np.float64).astype(np.float32)
    # count threshold in exp space: theta_i = exp(t_i - C); underflow to 0
    # only affects queries whose rank is huge (weight exactly 1) either way
    th32 = np.exp(t32.astype(np.float64) - C_SHIFT).astype(np.float32)

    q8 = q.astype(fp8)  # [B, D]
    p8 = p.astype(fp8)  # [P, D]
    # DRAM layout [KP, 2, 128, cols]: pair c plane kk holds rows
    # (2c+kk)*128 .. +127 of the transposed [D, cols] operand
    qT8 = np.ascontiguousarray(q8.T.reshape(KP, 2, 128, B))

    in_maps = []
    perms = []
    for c in range(NCORES):
        perm = _perm(c)
        perms.append(perm)
        qTc = np.ascontiguousarray(qT8[:, :, :, perm])
        pTc = np.ascontiguousarray(
            p8[c * PSLAB:(c + 1) * PSLAB].T.reshape(KP, 2, 128, PSLAB))
        thc = np.ascontiguousarray(th32[perm].reshape(MT, 128).T)
        in_maps.append({"qT": qTc, "pT": pTc, "thv": thc})
    return in_maps, t32, perms


def finalize(results, t32, perms):
    """Combine per-core partials into the scalar loss (fp64 host tail)."""
    se_tot = np.zeros(B, dtype=np.float64)
    cnt_tot = np.zeros(B, dtype=np.float64)
    for c in range(NCORES):
        perm = perms[c]
        # col m, row r -> query pi = m*128 + r; the last m-tile's stats
        # are split across cols MT-1 and MT (half-slab each)
        st = results[c]["st_out"].astype(np.float64)
        se, cnt = st[:, :MT + 1], st[:, MT + 1:]
        se[:, MT - 1] += se[:, MT]
        cnt[:, MT - 1] += cnt[:, MT]
        se_tot[perm] += se[:, :MT].T.ravel()
        cnt_tot[perm] += cnt[:, :MT].T.ravel()
    lse = C_SHIFT + np.log(se_tot)
    raw = lse - t32.astype(np.float64)
    w = 1.0 + ALPHA * np.exp(-((cnt_tot - OPTIMAL_RANK) ** 2)
                             / (2.0 * SIGMA ** 2))
    return np.float32(np.mean(raw * w))


def _get_nc():
    if "nc" not in _STATE:
        _STATE["nc"] = _build_nc()
    return _STATE["nc"]


def kernel(q_reps, p_reps, n_passages):
    assert int(np.asarray(n_passages)) == NP
    nc = _get_nc()
    in_maps, t32, perms = prepare(q_reps, p_reps)
    try:
        res = run_bass_kernel_spmd(nc, in_maps, core_ids=list(range(NCORES)))
    except Exception:
        # rare transient NRT_EXEC_UNIT_UNRECOVERABLE; reset the PJRT
        # client and retry once
        import time
        try:
            import jax
            jax.clear_caches()
            jax.extend.backend.clear_backends()
        except Exception:
            pass
        time.sleep(10)
        res = run_bass_kernel_spmd(nc, in_maps, core_ids=list(range(NCORES)))
    return finalize(res.results, t32, perms)


def run_profiled(q_reps, p_reps, n_passages, trace=True):
    """Same as kernel() but returns (loss, BassKernelResults) with NTFF
    profile (requires the antenv.axon_hooks shim; see _install_ntff_shim)."""
    nc = _get_nc()
    in_maps, t32, perms = prepare(q_reps, p_reps)
    res = run_bass_kernel_spmd(nc, in_maps, core_ids=list(range(NCORES)),
                               trace=trace)
    loss = finalize(res.results, t32, perms)
    return loss, res


def _install_ntff_shim():
    """Provide antenv.axon_hooks (absent in this image) so trace=True works."""
    import types
    import antenv
    if "antenv.axon_hooks" in sys.modules:
        return
    mod = types.ModuleType("antenv.axon_hooks")
    mod._hook = None
    mod.set_axon_ntff_profile_hook = lambda h: setattr(mod, "_hook", h)
    mod.get_axon_ntff_profile_hook = lambda: mod._hook
    sys.modules["antenv.axon_hooks"] = mod
    antenv.axon_hooks = mod
    try:
        from trn_agent_boot.trn_boot import _ntff_profile_via_ctypes
        hook = _ntff_profile_via_ctypes("/opt/axon/libaxon_pjrt.so")
        if hook is not None:
            mod._hook = hook
    except Exception:
        pass
